# revision 14
# baseline (speedup 1.0000x reference)
"""Trainium2 Bass kernel for nn_LossTDSurv (survival loss over hazards).

Strategy (v4): the loss is row-permutation invariant and only ever reads
columns 0..idx of each row, so the host sorts rows by idx and ships, in
bf16, just the needed prefix q_k = 2*(1-h_k) of every row.  All per-row
ragged sums become products of a compile-time-constant column prefix:

    prodA = prod_{k<=v-2} q_k      -> A = cond_sum = ln(prodA) - W*ln2
    logWt = ln(clip(1 - 2^-W*prodA, 1e-8))   (no exp/ln roundtrip)
    C_sum = sum A + sum ln(q_{v-1} q_v) - 2*ln2*rows

Group-to-core mapping: core c takes the 8 idx-groups {8s+c | s even} u
{8s+7-c | s odd}, exactly one per width-8 octave band, so EVERY core runs
the identical program with 8 fixed slot widths W_s' = 8(s+1)+2.  Groups
are right-aligned in their slot and padded left with the multiplicative
identity (2.0 after scaling), which keeps the product over the first
W = W_s'-2 columns correct for every v; h_idx, q_{idx-1}, q_idx are just
strided views of the same block (columns W, W+1).

The host packs q' = 2q because the ACT Ln spline saturates below ~1e-19
while prodA legitimately reaches e^-87; the 2^W prefactor keeps every Ln
input in the accurate range and the host subtracts the exact ln2
corrections from the affected sums.

Per-slot products use two levels of bf16 tensor_tensor (2x DVE perf
mode; all slices 4-byte aligned by construction) before a 1x
tensor_reduce(mult) over W/4 columns, cutting DVE element passes ~30%.
The Pool engine does the scalar fixups, the ACT engine does every Ln
with free row-sum accumulation.  Per-core HBM traffic ~5.3 MB.

Per-core output: [128, 12] fp32 partial sums; host combines in float64.
"""

import numpy as np

B_TOTAL = 524288
T = 64
N_CORES = 8
NSLOT = 8
PREFW = [8 * (s + 1) for s in range(NSLOT)]     # product prefix width W
WIDTHS = [w + 2 for w in PREFW]                 # slot width W' = W + 2
SLOT_ORDER = list(range(NSLOT - 1, -1, -1))     # process big slots first
DMA_CHUNKS = [[7], [6], [5], [4, 3], [2, 1, 0]]  # ~1MB each
HALves = ([7, 6, 5, 4], [3, 2, 1, 0])
CLIP_WT = 1e-8
CLIP_PA = 2e-17   # on the 2^W-scaled product, inside Ln's accurate range

_CACHE = {}


def _build_nc(jb):
    """Single-core SPMD Bass program (same NEFF on all 8 cores)."""
    import concourse.bacc as bacc
    import concourse.mybir as mybir
    import concourse.tile as tile

    f32 = mybir.dt.float32
    bf16 = mybir.dt.bfloat16
    AF = mybir.ActivationFunctionType
    OP = mybir.AluOpType
    AX = mybir.AxisListType

    nb = NSLOT * jb

    nc = bacc.Bacc("TRN2", target_bir_lowering=False, debug=False)

    lanew = jb * sum(WIDTHS)
    qpack = nc.dram_tensor("qpack", [128, lanew], bf16, kind="ExternalInput")
    eside = nc.dram_tensor("eside", [128, nb], bf16, kind="ExternalInput")
    partials = nc.dram_tensor("partials", [128, 12], f32, kind="ExternalOutput")

    # qpack column offset of each slot, in processing (descending) order
    qoff = {}
    off = 0
    for s in SLOT_ORDER:
        qoff[s] = off
        off += jb * WIDTHS[s]

    with tile.TileContext(nc) as tc:
        with (
            tc.tile_pool(name="io", bufs=3) as io,
            tc.tile_pool(name="work", bufs=1) as work,
            tc.tile_pool(name="pers", bufs=1) as pers,
        ):
            Eb = pers.tile([128, nb], bf16, tag="Eb")
            Ab = pers.tile([128, nb], f32, tag="Ab")
            T2 = pers.tile([128, nb], f32, tag="T2")
            Aln = pers.tile([128, nb], f32, tag="Aln")
            logwt = pers.tile([128, nb], f32, tag="logwt")
            loghv = pers.tile([128, nb], f32, tag="loghv")
            lgv = pers.tile([128, nb], f32, tag="lgv")
            Qb = pers.tile([128, nb], bf16, tag="Qb")
            scr = pers.tile([128, nb], bf16, tag="scr")
            scr2 = pers.tile([128, nb], bf16, tag="scr2")
            acc = pers.tile([128, 12], f32, tag="acc")

            nc.sync.dma_start(Eb[:], eside[:])
            nc.scalar.activation(scr2[:], Eb[:], AF.Copy,
                                 accum_out=acc[:, 11:12])        # s_e

            chunk_tiles = {}
            for chunk in DMA_CHUNKS:
                cw = sum(jb * WIDTHS[s] for s in chunk)
                Ct = io.tile([128, cw], bf16, tag="C")
                nc.sync.dma_start(
                    Ct[:], qpack[:, qoff[chunk[0]]:qoff[chunk[0]] + cw])
                for s in chunk:
                    chunk_tiles[s] = (Ct, qoff[s] - qoff[chunk[0]])

            for s in SLOT_ORDER:
                wp = WIDTHS[s]
                w = PREFW[s]
                sl = np.s_[:, s * jb:(s + 1) * jb]
                Ct, coff = chunk_tiles[s]
                blk = Ct[:, coff:coff + jb * wp].rearrange(
                    "p (j w) -> p j w", w=wp)
                # two bf16 pairwise-product levels (2x DVE mode), then 1x reduce
                P1 = work.tile([128, jb * (w // 2)], bf16, tag=f"P1_{s}")
                p1v = P1[:].rearrange("p (j w) -> p j w", w=w // 2)
                nc.vector.tensor_tensor(
                    out=p1v, in0=blk[:, :, 0:w // 2],
                    in1=blk[:, :, w // 2:w], op=OP.mult)
                P2 = work.tile([128, jb * (w // 4)], bf16, tag=f"P2_{s}")
                p2v = P2[:].rearrange("p (j w) -> p j w", w=w // 4)
                nc.vector.tensor_tensor(
                    out=p2v, in0=p1v[:, :, 0:w // 4],
                    in1=p1v[:, :, w // 4:w // 2], op=OP.mult)
                nc.vector.tensor_reduce(Ab[sl], p2v, axis=AX.X, op=OP.mult)

                # Pool: logWt arg 1 - 2^-W*prodA ; qq = q'_{v-1} q'_v = 4 qq
                nc.gpsimd.tensor_scalar(
                    out=T2[sl], in0=Ab[sl], scalar1=-(2.0 ** -w),
                    scalar2=1.0, op0=OP.mult, op1=OP.add)
                nc.gpsimd.tensor_tensor(
                    out=Qb[sl], in0=blk[:, :, w], in1=blk[:, :, w + 1],
                    op=OP.mult)
                # ACT: ln h_v = Ln(1 - q'_v/2) ; ln(1-h_v) = Ln(q'_v/2)
                nc.scalar.activation(loghv[sl], blk[:, :, w + 1], AF.Ln,
                                     bias=1.0, scale=-0.5)
                nc.scalar.activation(lgv[sl], blk[:, :, w + 1], AF.Ln,
                                     scale=0.5)

            for h, slots in enumerate(HALves):
                lo = min(slots) * jb
                hi = (max(slots) + 1) * jb
                hs = np.s_[:, lo:hi]
                a0 = 5 * h
                nc.gpsimd.tensor_scalar_max(out=T2[hs], in0=T2[hs],
                                            scalar1=CLIP_WT)
                nc.gpsimd.tensor_scalar_max(out=Ab[hs], in0=Ab[hs],
                                            scalar1=CLIP_PA)
                nc.scalar.activation(Aln[hs], Ab[hs], AF.Ln,
                                     accum_out=acc[:, a0:a0 + 1])  # s_a
                nc.scalar.activation(logwt[hs], T2[hs], AF.Ln)
                nc.vector.scalar_tensor_tensor(
                    out=Aln[hs], in0=Aln[hs], scalar=0.0, in1=Eb[hs],
                    op0=OP.add, op1=OP.mult,
                    accum_out=acc[:, a0 + 1:a0 + 2])               # s_eA
                nc.vector.scalar_tensor_tensor(
                    out=logwt[hs], in0=logwt[hs], scalar=0.0, in1=Eb[hs],
                    op0=OP.add, op1=OP.mult,
                    accum_out=acc[:, a0 + 2:a0 + 3])               # s_elogwt
                nc.vector.scalar_tensor_tensor(
                    out=loghv[hs], in0=loghv[hs], scalar=0.0, in1=Eb[hs],
                    op0=OP.add, op1=OP.mult,
                    accum_out=acc[:, a0 + 3:a0 + 4])               # s_eloghv
                nc.vector.scalar_tensor_tensor(
                    out=lgv[hs], in0=lgv[hs], scalar=0.0, in1=Eb[hs],
                    op0=OP.add, op1=OP.mult,
                    accum_out=acc[:, a0 + 4:a0 + 5])               # s_elgv

            nc.scalar.activation(scr[:], Qb[:], AF.Ln,
                                 accum_out=acc[:, 10:11])          # s_cq
            nc.sync.dma_start(partials[:], acc[:])

    nc.finalize()
    return nc


def _core_groups(c):
    return [8 * s + c if s % 2 == 0 else 8 * s + 7 - c for s in range(NSLOT)]


def _pack_core(c, q2, ev, rows_by_group, jb):
    """Pack one core's 8 groups: qpack [128, jb*sum(W')] bf16 (slots in
    descending order, right-aligned, pad 2.0) and eside [128, 8*jb]."""
    import ml_dtypes

    bf = ml_dtypes.bfloat16
    gr = 128 * jb
    nb = NSLOT * jb
    groups = _core_groups(c)
    qblocks = []
    e_all = np.zeros((128, nb), np.float32)

    for s in SLOT_ORDER:
        v = groups[s]
        wp = WIDTHS[s]
        rows = rows_by_group[v]
        n = len(rows)
        assert n <= gr, f"group {v} overflow: {n} > {gr}"
        blk = np.full((gr, wp), 2.0, np.float32)
        blk[:n, wp - v - 1:] = q2[rows, :v + 1]
        # dummy rows: the two extraction columns (q_{v-1}, q_v, outside the
        # product prefix) must be 1.0 so their logs are finite/zero
        blk[n:, wp - 2:] = 1.0
        qblocks.append(blk.reshape(128, jb * wp))

        e = np.zeros(gr, np.float32)
        e[:n] = ev[rows]
        e_all[:, s * jb:(s + 1) * jb] = e.reshape(128, jb)

    qpack = np.ascontiguousarray(np.concatenate(qblocks, axis=1)).astype(bf)
    return {"qpack": qpack, "eside": e_all.astype(bf)}


def _combine(partials_list, b_total, corr_a, corr_eA, corr_cq):
    s = np.zeros(12, np.float64)
    for pcore in partials_list:
        s += pcore.astype(np.float64).sum(axis=0)
    s_a = s[0] + s[5] - corr_a
    s_eA = s[1] + s[6] - corr_eA
    s_elogwt = s[2] + s[7]
    s_eloghv = s[3] + s[8]
    s_elgv = s[4] + s[9]
    s_cq = s[10] - corr_cq
    s_e = s[11]
    L_z = -(s_eloghv + s_eA) / s_e
    L_c = -(s_a - s_eA + s_elogwt) / b_total
    nll = -(s_a + s_cq + s_eloghv - s_elgv) / b_total
    return np.float32(0.5 * L_z + 0.5 * L_c + 1.0 * nll)


def kernel(preds: np.ndarray, target: np.ndarray) -> np.ndarray:
    from concourse.bass_utils import run_bass_kernel_spmd

    b_total = preds.shape[0]
    preds = np.asarray(preds, np.float32).reshape(b_total, T)
    target = np.asarray(target, np.float32).reshape(b_total, 3)
    idx = target[:, 0].astype(np.int64)
    ev = target[:, 1].astype(np.float32)
    q2 = np.float32(2.0) - np.float32(2.0) * preds   # 2q, exact bf16 scale

    counts = np.bincount(idx, minlength=T)
    jb = max(2, int(np.ceil(counts.max() / 128)))

    order = np.argsort(idx, kind="stable")
    rows_by_group = np.split(order, np.cumsum(counts)[:-1])

    if _CACHE.get("jb") != jb:
        _CACHE["nc"] = _build_nc(jb)
        _CACHE["jb"] = jb
    nc = _CACHE["nc"]

    in_maps = [_pack_core(c, q2, ev, rows_by_group, jb) for c in range(N_CORES)]

    # exact corrections for the host-side 2x scaling of q
    ln2 = float(np.log(2.0))
    w_row = 8.0 * (idx // 8 + 1)               # product prefix width per row
    corr_a = ln2 * 128 * jb * N_CORES * sum(PREFW)
    corr_eA = ln2 * float((ev.astype(np.float64) * w_row).sum())
    corr_cq = 2.0 * ln2 * b_total              # dummies contribute exactly 0

    res = run_bass_kernel_spmd(nc, in_maps, core_ids=list(range(N_CORES)))
    _CACHE["last_results"] = res
    return _combine([r["partials"] for r in res.results], float(b_total),
                    corr_a, corr_eA, corr_cq)


if __name__ == "__main__":
    pass


# revision 20
# speedup vs baseline: 1.3221x; 1.3221x over previous
"""Trainium2 Bass kernel for nn_LossTDSurv (survival loss over hazards).

Strategy (v4): the loss is row-permutation invariant and only ever reads
columns 0..idx of each row, so the host sorts rows by idx and ships, in
bf16, just the needed prefix q_k = 2*(1-h_k) of every row.  All per-row
ragged sums become products of a compile-time-constant column prefix:

    prodA = prod_{k<=v-2} q_k      -> A = cond_sum = ln(prodA) - W*ln2
    logWt = ln(clip(1 - 2^-W*prodA, 1e-8))   (no exp/ln roundtrip)
    C_sum = sum A + sum ln(q_{v-1} q_v) - 2*ln2*rows

Group-to-core mapping: core c takes the 8 idx-groups {8s+c | s even} u
{8s+7-c | s odd}, exactly one per width-8 octave band, so EVERY core runs
the identical program with 8 fixed slot widths W_s' = 8(s+1)+2.  Groups
are right-aligned in their slot and padded left with the multiplicative
identity (2.0 after scaling), which keeps the product over the first
W = W_s'-2 columns correct for every v; h_idx, q_{idx-1}, q_idx are just
strided views of the same block (columns W, W+1).

The host packs q' = 2q because the ACT Ln spline saturates below ~1e-19
while prodA legitimately reaches e^-87; the 2^W prefactor keeps every Ln
input in the accurate range and the host subtracts the exact ln2
corrections from the affected sums.

Per-slot products use two levels of bf16 tensor_tensor (2x DVE perf
mode; all slices 4-byte aligned by construction) before a 1x
tensor_reduce(mult) over W/4 columns, cutting DVE element passes ~30%.
The Pool engine does the scalar fixups, the ACT engine does every Ln
with free row-sum accumulation.  Per-core HBM traffic ~5.3 MB.

Per-core output: [128, 12] fp32 partial sums; host combines in float64.
"""

import numpy as np

B_TOTAL = 524288
T = 64
N_CORES = 8
NSLOT = 8
PREFW = [8 * (s + 1) for s in range(NSLOT)]     # product prefix width W
WIDTHS = [w + 2 for w in PREFW]                 # slot width W' = W + 2
SLOT_ORDER = list(range(NSLOT - 1, -1, -1))     # process big slots first
DMA_CHUNKS = [[7], [6], [5], [4, 3], [2, 1, 0]]  # ~1MB each
CLIP_WT = 1e-8
CLIP_PA = 2e-17   # on the 2^W-scaled product, inside Ln's accurate range

_CACHE = {}


def _build_nc(jb):
    """Single-core SPMD Bass program (same NEFF on all 8 cores)."""
    import concourse.bacc as bacc
    import concourse.mybir as mybir
    import concourse.tile as tile

    f32 = mybir.dt.float32
    bf16 = mybir.dt.bfloat16
    AF = mybir.ActivationFunctionType
    OP = mybir.AluOpType
    AX = mybir.AxisListType

    nb = NSLOT * jb

    nc = bacc.Bacc("TRN2", target_bir_lowering=False, debug=False)

    lanew = jb * sum(WIDTHS)
    qpack = nc.dram_tensor("qpack", [128, lanew], bf16, kind="ExternalInput")
    eside = nc.dram_tensor("eside", [128, nb], bf16, kind="ExternalInput")
    partials = nc.dram_tensor("partials", [128, 8], f32, kind="ExternalOutput")

    # qpack column offset of each slot, in processing (descending) order
    qoff = {}
    off = 0
    for s in SLOT_ORDER:
        qoff[s] = off
        off += jb * WIDTHS[s]

    with tile.TileContext(nc) as tc:
        with (
            tc.tile_pool(name="io", bufs=3) as io,
            tc.tile_pool(name="work", bufs=1) as work,
            tc.tile_pool(name="pers", bufs=1) as pers,
        ):
            Eb = pers.tile([128, nb], bf16, tag="Eb")
            Ab = pers.tile([128, nb], f32, tag="Ab")
            T2 = pers.tile([128, nb], f32, tag="T2")
            Aln = pers.tile([128, nb], f32, tag="Aln")
            logwt = pers.tile([128, nb], f32, tag="logwt")
            loghv = pers.tile([128, nb], f32, tag="loghv")
            lgv = pers.tile([128, nb], f32, tag="lgv")
            Qb = pers.tile([128, nb], bf16, tag="Qb")
            scr = pers.tile([128, nb], bf16, tag="scr")
            scr2 = pers.tile([128, nb], bf16, tag="scr2")
            acc = pers.tile([128, 8], f32, tag="acc")

            nc.sync.dma_start(Eb[:], eside[:])
            nc.vector.memset(acc[:, 7:8], 0.0)
            # s_e = sum e  (e*e == e); DVE so ACT only ever loads the Ln set
            nc.vector.scalar_tensor_tensor(
                out=scr2[:], in0=Eb[:], scalar=0.0, in1=Eb[:],
                op0=OP.add, op1=OP.mult, accum_out=acc[:, 6:7])

            chunk_tiles = {}
            for chunk in DMA_CHUNKS:
                cw = sum(jb * WIDTHS[s] for s in chunk)
                Ct = io.tile([128, cw], bf16, tag="C")
                nc.sync.dma_start(
                    Ct[:], qpack[:, qoff[chunk[0]]:qoff[chunk[0]] + cw])
                for s in chunk:
                    chunk_tiles[s] = (Ct, qoff[s] - qoff[chunk[0]])

            for s in SLOT_ORDER:
                wp = WIDTHS[s]
                w = PREFW[s]
                sl = np.s_[:, s * jb:(s + 1) * jb]
                Ct, coff = chunk_tiles[s]
                blk = Ct[:, coff:coff + jb * wp].rearrange(
                    "p (j w) -> p j w", w=wp)
                # bf16 pairwise-product tree (2x DVE mode) while the half
                # width stays even (4-byte-aligned split), then 1x reduce
                cur, cw = blk, w
                lvl = 0
                while cw % 4 == 0:   # keeps every split 4-byte aligned
                    hw = cw // 2
                    Pn = work.tile([128, jb * hw], bf16, tag=f"P{s}_{lvl}")
                    pv = Pn[:].rearrange("p (j w) -> p j w", w=hw)
                    nc.vector.tensor_tensor(
                        out=pv, in0=cur[:, :, 0:hw],
                        in1=cur[:, :, hw:2 * hw], op=OP.mult)
                    cur, cw, lvl = pv, hw, lvl + 1
                nc.vector.tensor_reduce(Ab[sl], cur, axis=AX.X, op=OP.mult)

                # ACT: logWt argument 1 - 2^-W*prodA (free affine + Identity)
                nc.scalar.activation(T2[sl], Ab[sl], AF.Identity,
                                     bias=1.0, scale=-(2.0 ** -w))
                # Pool: qq = q'_{v-1} q'_v = 4 q_{v-1} q_v
                nc.gpsimd.tensor_tensor(
                    out=Qb[sl], in0=blk[:, :, w], in1=blk[:, :, w + 1],
                    op=OP.mult)
                # ACT: ln h_v = Ln(1 - q'_v/2) ; ln(1-h_v) = Ln(q'_v/2)
                nc.scalar.activation(loghv[sl], blk[:, :, w + 1], AF.Ln,
                                     bias=1.0, scale=-0.5)
                nc.scalar.activation(lgv[sl], blk[:, :, w + 1], AF.Ln,
                                     scale=0.5)

            # --- epilogue (full width) ---
            nc.vector.tensor_scalar_max(out=T2[:], in0=T2[:], scalar1=CLIP_WT)
            nc.vector.tensor_scalar_max(out=Ab[:], in0=Ab[:], scalar1=CLIP_PA)
            nc.scalar.activation(Aln[:], Ab[:], AF.Ln,
                                 accum_out=acc[:, 0:1])            # s_a
            nc.scalar.activation(logwt[:], T2[:], AF.Ln)
            nc.scalar.activation(scr[:], Qb[:], AF.Ln,
                                 accum_out=acc[:, 5:6])            # s_cq
            nc.vector.scalar_tensor_tensor(
                out=Aln[:], in0=Aln[:], scalar=0.0, in1=Eb[:],
                op0=OP.add, op1=OP.mult, accum_out=acc[:, 1:2])    # s_eA
            nc.vector.scalar_tensor_tensor(
                out=logwt[:], in0=logwt[:], scalar=0.0, in1=Eb[:],
                op0=OP.add, op1=OP.mult, accum_out=acc[:, 2:3])    # s_elogwt
            nc.vector.scalar_tensor_tensor(
                out=loghv[:], in0=loghv[:], scalar=0.0, in1=Eb[:],
                op0=OP.add, op1=OP.mult, accum_out=acc[:, 3:4])    # s_eloghv
            nc.vector.scalar_tensor_tensor(
                out=lgv[:], in0=lgv[:], scalar=0.0, in1=Eb[:],
                op0=OP.add, op1=OP.mult, accum_out=acc[:, 4:5])    # s_elgv

            nc.sync.dma_start(partials[:], acc[:])

    nc.finalize()
    return nc


def _core_groups(c):
    return [8 * s + c if s % 2 == 0 else 8 * s + 7 - c for s in range(NSLOT)]


def _pack_core(c, q2, ev, rows_by_group, jb):
    """Pack one core's 8 groups: qpack [128, jb*sum(W')] bf16 (slots in
    descending order, right-aligned, pad 2.0) and eside [128, 8*jb]."""
    import ml_dtypes

    bf = ml_dtypes.bfloat16
    gr = 128 * jb
    nb = NSLOT * jb
    groups = _core_groups(c)
    qblocks = []
    e_all = np.zeros((128, nb), np.float32)

    for s in SLOT_ORDER:
        v = groups[s]
        wp = WIDTHS[s]
        rows = rows_by_group[v]
        n = len(rows)
        assert n <= gr, f"group {v} overflow: {n} > {gr}"
        blk = np.full((gr, wp), 2.0, np.float32)
        blk[:n, wp - v - 1:] = q2[rows, :v + 1]
        # dummy rows: the two extraction columns (q_{v-1}, q_v, outside the
        # product prefix) must be 1.0 so their logs are finite/zero
        blk[n:, wp - 2:] = 1.0
        qblocks.append(blk.reshape(128, jb * wp))

        e = np.zeros(gr, np.float32)
        e[:n] = ev[rows]
        e_all[:, s * jb:(s + 1) * jb] = e.reshape(128, jb)

    qpack = np.ascontiguousarray(np.concatenate(qblocks, axis=1)).astype(bf)
    return {"qpack": qpack, "eside": e_all.astype(bf)}


def _combine(partials_list, b_total, corr_a, corr_eA, corr_cq):
    s = np.zeros(8, np.float64)
    for pcore in partials_list:
        s += pcore.astype(np.float64).sum(axis=0)
    s_a = s[0] - corr_a
    s_eA = s[1] - corr_eA
    s_elogwt = s[2]
    s_eloghv = s[3]
    s_elgv = s[4]
    s_cq = s[5] - corr_cq
    s_e = s[6]
    L_z = -(s_eloghv + s_eA) / s_e
    L_c = -(s_a - s_eA + s_elogwt) / b_total
    nll = -(s_a + s_cq + s_eloghv - s_elgv) / b_total
    return np.float32(0.5 * L_z + 0.5 * L_c + 1.0 * nll)


def kernel(preds: np.ndarray, target: np.ndarray) -> np.ndarray:
    from concourse.bass_utils import run_bass_kernel_spmd

    b_total = preds.shape[0]
    preds = np.asarray(preds, np.float32).reshape(b_total, T)
    target = np.asarray(target, np.float32).reshape(b_total, 3)
    idx = target[:, 0].astype(np.int64)
    ev = target[:, 1].astype(np.float32)
    q2 = np.float32(2.0) - np.float32(2.0) * preds   # 2q, exact bf16 scale

    counts = np.bincount(idx, minlength=T)
    jb = max(2, int(np.ceil(counts.max() / 128)))

    order = np.argsort(idx, kind="stable")
    rows_by_group = np.split(order, np.cumsum(counts)[:-1])

    if _CACHE.get("jb") != jb:
        _CACHE["nc"] = _build_nc(jb)
        _CACHE["jb"] = jb
    nc = _CACHE["nc"]

    in_maps = [_pack_core(c, q2, ev, rows_by_group, jb) for c in range(N_CORES)]

    # exact corrections for the host-side 2x scaling of q
    ln2 = float(np.log(2.0))
    w_row = 8.0 * (idx // 8 + 1)               # product prefix width per row
    corr_a = ln2 * 128 * jb * N_CORES * sum(PREFW)
    corr_eA = ln2 * float((ev.astype(np.float64) * w_row).sum())
    corr_cq = 2.0 * ln2 * b_total              # dummies contribute exactly 0

    res = run_bass_kernel_spmd(nc, in_maps, core_ids=list(range(N_CORES)))
    _CACHE["last_results"] = res
    return _combine([r["partials"] for r in res.results], float(b_total),
                    corr_a, corr_eA, corr_cq)


if __name__ == "__main__":
    pass
